# revision 1
# baseline (speedup 1.0000x reference)
"""Two-layer GRU encoder (B=64, T=12, N=325, D=2, H=256) on 8 TRN2 NeuronCores.

v2.5: fp16 compute + fp8 (e4m3) DoubleRow matmuls on the z/r paths. Hidden
states live in fp16 tiles; packed e4m3 copies for the DoubleRow moving
operands are produced by DVE casts each step (h0's fp8 copy is shared by the
layer-0 recurrence and the layer-1 x-projection). Activations are fused
across PSUM banks (sigmoid over 4 banks, tanh over 2); layer-1 biases ride
K=1 ones-row matmuls so no activation needs a bias operand.

The whole run is ONE flat software pipeline over (t, layer, chunk) stages
with SKEW=3 between the z/r stage and the candidate stage. PSUM is managed
manually inside a single 8-bank tile: z/r stages alternate between two
4-bank regions so stage i+1 never waits on sigma(i); each candidate borrows
the z-banks of its own stage's region (free between sigma and the region's
next reuse). Subtile dependency tracking provides the hazards.
"""

import numpy as np
import ml_dtypes
from contextlib import ExitStack

import concourse.bass as bass
import concourse.tile as tile
from concourse import bacc, mybir
from concourse import bass_utils

F16 = np.float16
E4M3 = ml_dtypes.float8_e4m3fn
AF = mybir.ActivationFunctionType
DT = mybir.dt
DR = mybir.MatmulPerfMode.DoubleRow

H = 256
T = 12
B = 64
N = 325
D = 2
NCORES = 8
B_SH = B // NCORES            # 8
M = B_SH * N                  # 2600
_CWS = [434, 434, 434, 434, 432, 432]
CHUNKS = []
_o = 0
for _w in _CWS:
    CHUNKS.append((_o, _w))
    _o += _w
NCH = len(CHUNKS)
PW = 448                      # padded half stride in fp16 h tiles
SKEW = 3

_CACHE = {}


def _build_nc():
    nc = bacc.Bacc("TRN2", target_bir_lowering=False, debug=False,
                   enable_asserts=False)
    f16 = DT.float16
    f8 = DT.float8e4
    f32 = DT.float32

    xt_d = nc.dram_tensor("xt", (3, T * M), f16, kind="ExternalInput").ap()
    wx0_d = nc.dram_tensor("wx0", (3, 768), f16, kind="ExternalInput").ap()
    whzr0_d = nc.dram_tensor("whzr0", (128, 1024), f8, kind="ExternalInput").ap()
    whh0_d = nc.dram_tensor("whh0", (128, 512), f16, kind="ExternalInput").ap()
    wx1zr_d = nc.dram_tensor("wx1zr", (128, 1024), f8, kind="ExternalInput").ap()
    wx1c_d = nc.dram_tensor("wx1c", (128, 512), f16, kind="ExternalInput").ap()
    whzr1_d = nc.dram_tensor("whzr1", (128, 1024), f8, kind="ExternalInput").ap()
    whh1_d = nc.dram_tensor("whh1", (128, 512), f16, kind="ExternalInput").ap()
    bias1_d = nc.dram_tensor("bias1", (1, 768), f16, kind="ExternalInput").ap()
    out_d = nc.dram_tensor("out", (2, NCH, 128, 2 * PW), f16,
                           kind="ExternalOutput").ap()

    with tile.TileContext(nc) as tc, ExitStack() as ctx:
        const = ctx.enter_context(tc.tile_pool(name="const", bufs=1))
        hpool = ctx.enter_context(tc.tile_pool(name="hstate", bufs=1))
        work = ctx.enter_context(tc.tile_pool(name="work", bufs=4))
        psum = ctx.enter_context(tc.tile_pool(name="psum", bufs=1, space="PSUM"))

        def load(name, dram, shape, dtype):
            t_ = const.tile(list(shape), dtype, tag=name, name=name)
            nc.sync.dma_start(t_[:], dram[:])
            return t_

        wx0 = load("wx0", wx0_d, (3, 768), f16)
        xt = const.tile([3, T * M], f16, tag="xt", name="xt")
        for _t in range(T):
            nc.sync.dma_start(xt[:, _t * M:(_t + 1) * M],
                              xt_d[:, _t * M:(_t + 1) * M])
        whzr0 = load("whzr0", whzr0_d, (128, 1024), f8)
        whh0 = load("whh0", whh0_d, (128, 512), f16)
        wx1zr = load("wx1zr", wx1zr_d, (128, 1024), f8)
        wx1c = load("wx1c", wx1c_d, (128, 512), f16)
        whzr1 = load("whzr1", whzr1_d, (128, 1024), f8)
        whh1 = load("whh1", whh1_d, (128, 512), f16)
        bias1 = load("bias1", bias1_d, (1, 768), f16)
        ones = const.tile([1, 512], f16, tag="ones", name="ones")
        nc.vector.memset(ones[:], 1.0)

        # single 8-bank PSUM tile, manually banked
        pp8 = psum.tile([128, 8, 512], f32, tag="pp8", name="pp8", bufs=1)

        # fp16 hidden states, halves at [0:mw] and [PW:PW+mw]
        hst = {}
        h8st = {}
        for L in (0, 1):
            for ci in range(NCH):
                for pp in (0, 1):
                    nm = f"h{L}_{ci}_{pp}"
                    hst[(L, ci, pp)] = hpool.tile([128, 2 * PW], f16,
                                                  tag=nm, name=nm)
                    nm8 = f"h8_{L}_{ci}_{pp}"
                    h8st[(L, ci, pp)] = hpool.tile([128, 1024], f8,
                                                   tag=nm8, name=nm8)

        def h_f16(tile_, mw):  # [128, 2, mw] fp16 view (halves at stride PW)
            return tile_[:, :].rearrange("p (k m) -> p k m", k=2)[:, :, 0:mw]

        def h8_v(tile_, mw):  # [128, 2, mw] packed e4m3 view (stride 512)
            return tile_[:, :].rearrange("p (k m) -> p k m", k=2)[:, :, 0:mw]

        def wdr(w, g):  # [128, 2, 128] DR weight view for gate-half g
            return w[:, g * 256:(g + 1) * 256].rearrange("p (k f) -> p k f", k=2)

        # weight gate order: cols [z | r]; banks in region: [za zb ra rb]
        def emit_zr(nc_, t, L, ci, reg, s_zr):
            m0, mw = CHUNKS[ci]
            first = t == 0
            pp_r = 1 - t % 2
            # r-gate banks (reg+2, reg+3) first: they only wait sigma(i-2);
            # z-banks (reg+0, reg+1) last: they also wait the tanh of the
            # cand stage that borrowed them one slot ago.
            border = (2, 3, 0, 1)

            def dr_block(w, h8t, start):
                for g in border:
                    nc_.tensor.matmul(pp8[:, reg + g, 0:mw], wdr(w, g),
                                      h8_v(h8t, mw), start=start, stop=False,
                                      perf_mode=DR)

            if L == 0:
                if not first:
                    dr_block(whzr0, h8st[(0, ci, pp_r)], True)
                x_rhs = xt[:, t * M + m0: t * M + m0 + mw]
                for g in border:
                    nc_.tensor.matmul(pp8[:, reg + g, 0:mw],
                                      wx0[:, g * 128:(g + 1) * 128],
                                      x_rhs, start=first, stop=True)
            else:
                h08n = h8st[(0, ci, t % 2)]
                if not first:
                    dr_block(whzr1, h8st[(1, ci, pp_r)], True)
                dr_block(wx1zr, h08n, first)
                for g in border:
                    nc_.tensor.matmul(pp8[:, reg + g, 0:mw],
                                      bias1[:, g * 128:(g + 1) * 128],
                                      ones[:, 0:mw], start=False, stop=True)
            # fused sigmoid over the region's 4 banks -> s_zr [za zb ra rb]
            nc_.scalar.activation(
                s_zr[:, :].rearrange("p (g m) -> p g m", g=4)[:, :, 0:mw],
                pp8[:, reg:reg + 4, 0:mw], AF.Sigmoid)

        def emit_cand(nc_, t, L, ci, reg, s_zr, c):
            # c banks = z-banks (reg+0, reg+1) of this stage's own region
            m0, mw = CHUNKS[ci]
            first = t == 0
            pp_r = 1 - t % 2
            pp_w = t % 2
            hp = hst[(L, ci, pp_r)]
            hn = hst[(L, ci, pp_w)]
            rh = None
            if not first:
                rh = work.tile([128, 2 * PW], DT.float16, tag="rh",
                               name=f"rh{L}{ci}")
                s_r = s_zr[:, 2 * PW:].rearrange("p (k m) -> p k m", k=2)[:, :, 0:mw]
                nc_.vector.tensor_mul(h_f16(rh, mw), s_r, h_f16(hp, mw))
            whh = whh0 if L == 0 else whh1
            if L == 0:
                x_rhs = xt[:, t * M + m0: t * M + m0 + mw]
                for g in range(2):
                    nc_.tensor.matmul(pp8[:, reg + g, 0:mw],
                                      wx0[:, 512 + g * 128: 512 + (g + 1) * 128],
                                      x_rhs, start=True, stop=first)
            else:
                h0n = hst[(0, ci, pp_w)]
                for g in range(2):
                    for k in range(2):
                        nc_.tensor.matmul(
                            pp8[:, reg + g, 0:mw],
                            wx1c[:, k * 256 + g * 128: k * 256 + (g + 1) * 128],
                            h0n[:, k * PW:k * PW + mw],
                            start=(k == 0), stop=False)
                for g in range(2):
                    nc_.tensor.matmul(pp8[:, reg + g, 0:mw],
                                      bias1[:, 512 + g * 128: 512 + (g + 1) * 128],
                                      ones[:, 0:mw], start=False, stop=first)
            if not first:
                for g in range(2):
                    for k in range(2):
                        nc_.tensor.matmul(
                            pp8[:, reg + g, 0:mw],
                            whh[:, k * 256 + g * 128: k * 256 + (g + 1) * 128],
                            rh[:, k * PW:k * PW + mw],
                            start=False, stop=(k == 1))
            # fused tanh over the 2 borrowed banks
            nc_.scalar.activation(
                c[:, :].rearrange("p (g m) -> p g m", g=2)[:, :, 0:mw],
                pp8[:, reg:reg + 2, 0:mw], AF.Tanh)
            # blend
            s_z = s_zr[:, 0:2 * PW].rearrange("p (k m) -> p k m", k=2)[:, :, 0:mw]
            cv = c[:, :].rearrange("p (k m) -> p k m", k=2)[:, :, 0:mw]
            if first:
                nc_.vector.tensor_mul(h_f16(hn, mw), s_z, cv)
            else:
                d = work.tile([128, 2 * PW], DT.float16, tag="d", name=f"d{L}{ci}")
                nc_.vector.tensor_sub(h_f16(d, mw), cv, h_f16(hp, mw))
                zd = work.tile([128, 2 * PW], DT.float16, tag="zd", name=f"zd{L}{ci}")
                nc_.vector.tensor_mul(h_f16(zd, mw), s_z, h_f16(d, mw))
                nc_.vector.tensor_add(h_f16(hn, mw), h_f16(hp, mw), h_f16(zd, mw))
            # packed e4m3 copy for next-step DR reads (and L1 xp for L==0);
            # dead at the last step for L==1
            if L == 0 or t < T - 1:
                h8n = h8st[(L, ci, pp_w)]
                nc_.vector.tensor_copy(h8n[:, 0:mw], hn[:, 0:mw])
                nc_.vector.tensor_copy(h8n[:, 512:512 + mw], hn[:, PW:PW + mw])

        stages = [(t, L, ci) for t in range(T) for L in (0, 1)
                  for ci in range(NCH)]
        pending = {}
        for si in range(len(stages) + SKEW):
            if si < len(stages):
                t, L, ci = stages[si]
                s_zr = work.tile([128, 4 * PW], DT.float16, tag="szr",
                                 name=f"szr{L}{t}{ci}", bufs=SKEW + 2)
                emit_zr(nc, t, L, ci, 4 * (si % 2), s_zr)
                pending[si] = s_zr
            if si >= SKEW:
                sj = si - SKEW
                t, L, cj = stages[sj]
                c = work.tile([128, 2 * PW], DT.float16, tag="c",
                              name=f"c{L}{t}{cj}")
                emit_cand(nc, t, L, cj, 4 * (sj % 2), pending.pop(sj), c)

        ppf = (T - 1) % 2
        for L in (0, 1):
            for ci in range(NCH):
                nc.sync.dma_start(out_d[L, ci], hst[(L, ci, ppf)][:])

    nc.compile()
    return nc


def _prep_weights(inputs):
    def f32(x):
        return np.asarray(x, np.float32)

    def q8c(x):
        return np.clip(f32(x), -240, 240).astype(E4M3)

    def dr_pack(W):  # (256, G*128) -> (128, G*256) DR layout
        G = W.shape[1] // 128
        out = np.zeros((128, G * 256), np.float32)
        for g in range(G):
            for k in range(2):
                out[:, g * 256 + k * 128:g * 256 + (k + 1) * 128] = \
                    W[k * 128:(k + 1) * 128, g * 128:(g + 1) * 128]
        return out

    def kstack(W):  # (256, C) -> (128, 2C)
        return np.concatenate([W[:128], W[128:]], axis=1)

    ball = {}
    for L in (0, 1):
        bx = f32(inputs[f"bx{L}"])
        bhzr = f32(inputs[f"bhzr{L}"])
        bhh = f32(inputs[f"bhh{L}"])
        ball[L] = np.concatenate([bx[:2 * H] + bhzr, bx[2 * H:] + bhh])

    wx0 = np.concatenate([f32(inputs["Wx0"]), ball[0][None, :]], axis=0)
    wx1 = f32(inputs["Wx1"])
    return {
        "wx0": wx0.astype(F16),
        "whzr0": q8c(dr_pack(f32(inputs["Whzr0"]))),
        "whh0": kstack(f32(inputs["Whh0"])).astype(F16),
        "wx1zr": q8c(dr_pack(wx1[:, :2 * H])),
        "wx1c": kstack(wx1[:, 2 * H:]).astype(F16),
        "whzr1": q8c(dr_pack(f32(inputs["Whzr1"]))),
        "whh1": kstack(f32(inputs["Whh1"])).astype(F16),
        "bias1": ball[1][None, :].astype(F16),
    }


def kernel(**inputs):
    X = np.asarray(inputs["X"], np.float32)
    shared = _prep_weights(inputs)

    if "nc" not in _CACHE:
        _CACHE["nc"] = _build_nc()
    nc = _CACHE["nc"]

    in_maps = []
    ones = np.ones((1, T * M), np.float32)
    for c in range(NCORES):
        Xc = X[c * B_SH:(c + 1) * B_SH]                      # (8, T, N, D)
        xt = np.ascontiguousarray(Xc.transpose(3, 1, 0, 2)).reshape(D, T * M)
        m = dict(shared)
        m["xt"] = np.concatenate([xt, ones], axis=0).astype(F16)
        in_maps.append(m)
    _CACHE["in_maps"] = in_maps

    res = bass_utils.run_bass_kernel_spmd(nc, in_maps, core_ids=list(range(NCORES)))

    out = np.empty((2, B, N, H), np.float32)
    for c in range(NCORES):
        arr = np.asarray(res.results[c]["out"], dtype=np.float32)  # (2,6,128,2PW)
        per_core = np.empty((2, M, H), np.float32)
        for ci, (m0, mw) in enumerate(CHUNKS):
            blk = np.stack([arr[:, ci, :, 0:mw], arr[:, ci, :, PW:PW + mw]], axis=2)
            per_core[:, m0:m0 + mw, :] = blk.transpose(0, 3, 2, 1).reshape(2, mw, H)
        out[:, c * B_SH:(c + 1) * B_SH] = per_core.reshape(2, B_SH, N, H)
    return out

